# revision 2
# baseline (speedup 1.0000x reference)
"""BondReactivityPredictor Trainium2 kernel v2.

Sharding: edges (E=400000) split data-parallel across 8 NeuronCores
(50000/core, padded to 51200 = 25 groups x 2048). node_embedding +
atom logits live in a replicated [N, 256] f16 table ([emb(128) |
logit | pad]); each core fetches the rows for its edge shard with
DMAGatherAnt transpose-gathers (feature-major output, no on-chip
transposes). All matmuls run in f16 (f32 PSUM accumulate); LayerNorm
is computed with mean-centering folded into the weights (W' = W*C so
the matmul output is already centered), a J/128 matmul producing the
broadcast variance directly, and an fp16 quake+Newton rsqrt on DVE
(no activation-table switches: only Silu/Tanh/Square/Identity, all in
the silu table). sigmoid(x) = 0.5 + 0.5*tanh(x/2) is folded into the
f1 weights/bias so tanh rows feed the head directly.
"""

import os
import sys

import numpy as np

for _p in ("/opt/trn_rl_repo", "/root/.axon_site/_ro/trn_rl_repo"):
    if os.path.isdir(_p) and _p not in sys.path:
        sys.path.insert(0, _p)

import concourse.bass as bass
import concourse.bacc as bacc
import concourse.mybir as mybir
import concourse.tile as tile
from concourse import library_config
from concourse.masks import make_identity

F16 = mybir.dt.float16
F32 = mybir.dt.float32
I16 = mybir.dt.int16
AF = mybir.ActivationFunctionType
ALU = mybir.AluOpType
AX = mybir.AxisListType

N_NODES = 25000
D_IN = 16
D_EH = 128
D_H = 256
D_DUAL = 128
P = 128
T = 512                 # edges per macro tile
GROUP_E = 2048          # edges per gather group
MACROS = GROUP_E // T

N_CORES = 8
E_FULL = 400000
E_CORE = E_FULL // N_CORES
G_FULL = 25
E_PAD = G_FULL * GROUP_E

AUG_COLS = 256          # f16: [emb 128 | logit | 127 pad] = 512B rows
EPS = 1e-5
QMAGIC = 22971          # fp16 rsqrt magic: r0 = magic - (bits >> 1)
NEWTON = 1
GATHER_NIDX = 512       # idxs per DMAGatherAnt instr
GATHER_SP = True        # single_packet (safe at 512 idx)

WEIGHT_NAMES = (
    "W_be1", "b_be1", "g_be1", "bb_be1", "W_be2", "b_be2", "g_be2", "bb_be2",
    "W_be3", "b_be3", "g_be3", "bb_be3", "W_d1", "b_d1", "W_d2", "b_d2",
    "W_do", "b_do", "W_f1", "b_f1", "W_f2", "b_f2", "W_bo", "b_bo")


def build_program(G, repeats=1, flags=None):
    """Per-core program over G groups of 2048 edges.

    flags: zero/identity detection for bias/affine fast paths (computed
    from the actual inputs by kernel(); all-False is the general path).
    """
    fl = {"be_b0": True, "be_aff1": True, "d1_b0": True, "d2_b0": True,
          "f2_b0": True}
    if flags:
        fl.update(flags)

    nc = bacc.Bacc()

    aug = nc.declare_dram_parameter("aug", [N_NODES, AUG_COLS], F16, isOutput=False)
    ea_d = nc.declare_dram_parameter("ea_t", [D_IN, G * GROUP_E], F16, isOutput=False)
    dual_d = nc.declare_dram_parameter("dual_t", [D_DUAL, G * GROUP_E], F16, isOutput=False)
    srcw = nc.declare_dram_parameter("src_w", [128, G * GROUP_E // 16], I16, isOutput=False)
    dstw = nc.declare_dram_parameter("dst_w", [128, G * GROUP_E // 16], I16, isOutput=False)
    qc = nc.declare_dram_parameter("qconst", [128, 3], I16, isOutput=False)

    w_in = {}
    for name, shape in [
        ("W_be1", [D_IN, D_EH]), ("W_be2", [D_EH, D_EH]), ("W_be3", [D_EH, D_EH]),
        ("W_d1", [D_DUAL, D_H]), ("W_d2", [D_H, D_H]), ("W_do", [D_H, 1]),
        ("W_f1", [2 * P + D_EH + 3, D_H]), ("W_f2", [D_H, D_H]), ("W_bo", [D_H, 1]),
        ("b_be1", [D_EH]), ("g_be1", [D_EH]), ("bb_be1", [D_EH]),
        ("b_be2", [D_EH]), ("g_be2", [D_EH]), ("bb_be2", [D_EH]),
        ("b_be3", [D_EH]), ("g_be3", [D_EH]), ("bb_be3", [D_EH]),
        ("b_d1", [D_H]), ("b_d2", [D_H]), ("b_do", [1]),
        ("b_f1", [D_H]), ("b_f2", [D_H]), ("b_bo", [1]),
    ]:
        w_in[name] = nc.declare_dram_parameter(name, shape, F32, isOutput=False)

    out_d = nc.declare_dram_parameter("out", [G * GROUP_E], F32, isOutput=True)
    out_v = out_d.rearrange("(g e) -> g e", e=GROUP_E)

    with tile.TileContext(nc) as tc:
        with (
            tc.tile_pool(name="const", bufs=1) as const,
            tc.tile_pool(name="wst", bufs=1) as wst,
            tc.tile_pool(name="io", bufs=2) as io,
            tc.tile_pool(name="gat", bufs=2) as gat,
            tc.tile_pool(name="act", bufs=3) as act,
            tc.tile_pool(name="ps_y", bufs=2, space="PSUM") as ps_y,
            tc.tile_pool(name="ps_v", bufs=1, space="PSUM") as ps_v,
            tc.tile_pool(name="ps_b", bufs=2, space="PSUM") as ps_b,
            tc.tile_pool(name="ps_s", bufs=1, space="PSUM") as ps_s,
        ):
            nc.gpsimd.load_library(library_config.mlp)

            ident = const.tile([P, P], F32)
            make_identity(nc, ident[:])
            j128 = const.tile([P, P], F16)
            nc.vector.memset(j128[:], 1.0 / 128)
            qcol = const.tile([128, 3], I16)
            nc.sync.dma_start(qcol[:], qc[:, :])
            qmagic = const.tile([P, T], I16)
            nc.vector.memset(qmagic[:], QMAGIC)
            ones3 = const.tile([3, 1], F32)
            nc.vector.memset(ones3[:], 1.0)

            def cast_load(name, shape, src_ap):
                """f32 DRAM -> f16 SBUF (SWDGE casts in flight)."""
                t = const.tile(shape, F16, name=name)
                nc.gpsimd.dma_start(t[:], src_ap)
                return t

            def center_load(name, K, src_ap):
                """W' = W - rowmean(W): folds LN mean-centering into W."""
                stage = wst.tile([K, P], F32, tag=f"st_{name}")
                nc.sync.dma_start(stage[:], src_ap)
                s = wst.tile([K, 1], F32, tag=f"s_{name}")
                nc.vector.reduce_sum(s[:], stage[:], axis=AX.X)
                m = wst.tile([K, 1], F32, tag=f"m_{name}")
                nc.vector.tensor_scalar_mul(m[:], s[:], 1.0 / 128)
                w = const.tile([K, P], F16, name=name)
                nc.vector.tensor_scalar(
                    out=w[:], in0=stage[:], scalar1=m[:, 0:1], scalar2=None,
                    op0=ALU.subtract)
                return w

            wbe = [center_load(f"wbe{l}", D_IN if l == 1 else D_EH,
                               w_in[f"W_be{l}"][:, :]) for l in (1, 2, 3)]
            wd1 = cast_load("wd1", [P, D_H], w_in["W_d1"][:, :])
            wd2k = [cast_load(f"wd2_{ki}", [P, D_H],
                              w_in["W_d2"][ki * P:(ki + 1) * P, :]) for ki in range(2)]
            wdok = [cast_load(f"wdo_{ki}", [P, 1],
                              w_in["W_do"][ki * P:(ki + 1) * P, :]) for ki in range(2)]
            wa = cast_load("wa", [P, D_H], w_in["W_f1"][0:P, :])
            wb = cast_load("wb", [P, D_H], w_in["W_f1"][P:2 * P, :])
            wc = cast_load("wc", [P, D_H], w_in["W_f1"][2 * P:3 * P, :])
            wf2k = [cast_load(f"wf2_{ki}", [P, D_H],
                              w_in["W_f2"][ki * P:(ki + 1) * P, :]) for ki in range(2)]
            wbok = [cast_load(f"wbo_{ki}", [P, 1],
                              w_in["W_bo"][ki * P:(ki + 1) * P, :]) for ki in range(2)]

            # tail rows (tanh-encoded): weights 0.5*W_f1[384:387];
            # bias fold b_f1'' = b_f1 + 0.5*sum(W_f1[384:387]) as [P,2] col.
            wt_st = wst.tile([3, D_H], F32, tag="wt_st")
            nc.sync.dma_start(wt_st[:], w_in["W_f1"][3 * P:3 * P + 3, :])
            wtail = const.tile([3, D_H], F16, name="wtail")
            nc.vector.tensor_scalar_mul(wtail[:], wt_st[:], 0.5)
            ssum = ps_s.tile([1, D_H], F32, tag="sm")
            nc.tensor.matmul(ssum[:], ones3[:], wt_st[:], start=True, stop=True)
            bf1r = wst.tile([1, D_H], F32, tag="bf1r")
            nc.sync.dma_start(bf1r[:], w_in["b_f1"][None, :])
            bf1pp = wst.tile([1, D_H], F32, tag="bf1pp")
            nc.vector.tensor_scalar_mul(bf1pp[:], ssum[:], 0.5)
            nc.vector.tensor_add(bf1pp[:], bf1pp[:], bf1r[:])
            bf1c = const.tile([P, 2], F32, name="bf1c")
            for mc in range(2):
                tp = ps_s.tile([P, 1], F32, tag="sm")
                nc.tensor.transpose(tp[:], bf1pp[0:1, mc * P:(mc + 1) * P],
                                    ident[0:1, 0:1])
                nc.vector.tensor_copy(bf1c[:, mc:mc + 1], tp[:])

            def col2(name):
                t = const.tile([P, 2], F32, name=f"c_{name}")
                nc.sync.dma_start(t[:], w_in[name][:].rearrange("(mc d) -> d mc", d=P))
                return t

            bd1c = None if fl["d1_b0"] else col2("b_d1")
            bd2c = None if fl["d2_b0"] else col2("b_d2")
            bf2c = None if fl["f2_b0"] else col2("b_f2")

            def col1(name):
                t = const.tile([P, 1], F32, name=f"c_{name}")
                nc.sync.dma_start(t[:], w_in[name][:, None])
                return t

            becols = {}
            if not fl["be_aff1"]:
                for l in (1, 2, 3):
                    becols[("g", l)] = col1(f"g_be{l}")
                    becols[("bb", l)] = col1(f"bb_be{l}")
            if not fl["be_b0"]:
                # b' = b - mean(b) column, per layer
                onec = const.tile([P, 1], F32, name="onec")
                nc.vector.memset(onec[:], 1.0)
                oner = const.tile([1, P], F32, name="oner")
                nc.vector.memset(oner[:], 1.0)
                for l in (1, 2, 3):
                    bcol = col1(f"b_be{l}")
                    s11 = ps_s.tile([1, 1], F32, tag="sm")
                    nc.tensor.matmul(s11[:], bcol[:], onec[:], start=True, stop=True)
                    sc = wst.tile([1, 1], F32, tag=f"bs_{l}")
                    nc.vector.tensor_scalar_mul(sc[:], s11[:], 1.0 / 128)
                    mb = ps_s.tile([P, 1], F32, tag="sm")
                    nc.tensor.matmul(mb[:], oner[:], sc[:], start=True, stop=True)
                    bp = const.tile([P, 1], F32, name=f"bp_{l}")
                    nc.vector.tensor_sub(bp[:], bcol[:], mb[:])
                    becols[("b", l)] = bp

            bdoh = const.tile([1, 1], F32, name="bdoh")
            tmp11 = wst.tile([1, 1], F32, tag="t11")
            nc.sync.dma_start(tmp11[:], w_in["b_do"][None, :])
            nc.vector.tensor_scalar_mul(bdoh[:], tmp11[:], 0.5)
            bbo1 = const.tile([1, 1], F32, name="bbo1")
            nc.sync.dma_start(bbo1[:], w_in["b_bo"][None, :])

            # ---------------- main loop ----------------
            IDX_COLS = GATHER_NIDX // 16
            GPI = GROUP_E // GATHER_NIDX  # gathers per group per side

            def ln_layer(l, w, x_rhs):
                yc = ps_y.tile([P, T], F32, tag="y")
                nc.tensor.matmul(yc[:], w[:], x_rhs, start=True, stop=True)
                sqb = 0.0 if fl["be_b0"] else becols[("b", l)][:, 0:1]
                if fl["be_b0"]:
                    ysrc = yc
                else:
                    ysb = act.tile([P, T], F16, tag="ysb")
                    nc.vector.tensor_scalar_add(ysb[:], yc[:], sqb)
                    ysrc = ysb
                sq = act.tile([P, T], F16, tag="sq")
                if fl["be_b0"]:
                    nc.scalar.activation(sq[:], ysrc[:], AF.Square)
                else:
                    nc.vector.tensor_mul(sq[:], ysrc[:], ysrc[:])
                var = ps_v.tile([P, T], F32, tag="v")
                nc.tensor.matmul(var[:], j128[:], sq[:], start=True, stop=True)
                v16 = act.tile([P, T], F16, tag="v16")
                nc.vector.tensor_scalar_add(v16[:], var[:], EPS)
                qt = act.tile([P, T], I16, tag="qt")
                nc.vector.tensor_scalar(
                    out=qt[:], in0=v16[:].bitcast(I16), scalar1=qcol[:, 0:1],
                    scalar2=None, op0=ALU.logical_shift_right)
                r = act.tile([P, T], F16, tag="qr")
                nc.vector.tensor_tensor(
                    out=r[:].bitcast(I16), in0=qmagic[:], in1=qt[:],
                    op=ALU.subtract)
                for _ in range(NEWTON):
                    rr = act.tile([P, T], F16, tag="qrr")
                    nc.vector.tensor_mul(rr[:], r[:], r[:])
                    nc.vector.tensor_mul(rr[:], rr[:], v16[:])
                    nc.vector.tensor_scalar(
                        out=rr[:], in0=rr[:], scalar1=-0.5, scalar2=1.5,
                        op0=ALU.mult, op1=ALU.add)
                    r2 = act.tile([P, T], F16, tag="qr2")
                    nc.vector.tensor_mul(r2[:], r[:], rr[:])
                    r = r2
                xn = act.tile([P, T], F16, tag="xn")
                nc.vector.tensor_mul(xn[:], ysrc[:], r[:])
                x = act.tile([P, T], F16, tag=f"x{l}")
                if fl["be_aff1"]:
                    nc.scalar.activation(x[:], xn[:], AF.Silu)
                else:
                    nc.scalar.activation(x[:], xn[:], AF.Silu,
                                         bias=becols[("bb", l)][:, 0:1],
                                         scale=becols[("g", l)][:, 0:1])
                return x

            def group_body(g):
                sit = io.tile([128, GROUP_E // 16], I16, tag="sit")
                nc.sync.dma_start(sit[:], srcw[:, g * (GROUP_E // 16):(g + 1) * (GROUP_E // 16)])
                dit = io.tile([128, GROUP_E // 16], I16, tag="dit")
                nc.sync.dma_start(dit[:], dstw[:, g * (GROUP_E // 16):(g + 1) * (GROUP_E // 16)])
                ea_t = io.tile([D_IN, GROUP_E], F16, tag="ea")
                nc.sync.dma_start(ea_t[:], ea_d[:, g * GROUP_E:(g + 1) * GROUP_E])
                du_t = io.tile([D_DUAL, GROUP_E], F16, tag="du")
                nc.sync.dma_start(du_t[:], dual_d[:, g * GROUP_E:(g + 1) * GROUP_E])
                ob = io.tile([1, GROUP_E], F32, tag="ob")

                for mi in range(MACROS):
                    sl = slice(mi * T, (mi + 1) * T)
                    ics = slice(mi * IDX_COLS, (mi + 1) * IDX_COLS)
                    sg = gat.tile([128, 2, T], F16, tag="sg", bufs=3)
                    nc.gpsimd.dma_gather(
                        sg[:], aug[:, :], sit[:, ics], GATHER_NIDX,
                        GATHER_NIDX, AUG_COLS, transpose=True,
                        single_packet=GATHER_SP)
                    dg = gat.tile([128, 2, T], F16, tag="dg", bufs=3)
                    nc.gpsimd.dma_gather(
                        dg[:], aug[:, :], dit[:, ics], GATHER_NIDX,
                        GATHER_NIDX, AUG_COLS, transpose=True,
                        single_packet=GATHER_SP)

                    # BondEmbedding chain
                    x = ea_t[:, sl]
                    for l in (1, 2, 3):
                        x = ln_layer(l, wbe[l - 1], x)
                    x3 = x

                    # dual chain
                    d1p = ps_b.tile([P, 2, T], F32, tag="big")
                    for mc in range(2):
                        nc.tensor.matmul(d1p[:, mc, :], wd1[:, mc * P:(mc + 1) * P],
                                         du_t[:, sl], start=True, stop=True)
                    d1 = act.tile([P, 2, T], F16, tag="d1")
                    if fl["d1_b0"]:
                        nc.scalar.activation(d1[:], d1p[:], AF.Silu)
                    else:
                        for mc in range(2):
                            nc.scalar.activation(d1[:, mc, :], d1p[:, mc, :],
                                                 AF.Silu, bias=bd1c[:, mc:mc + 1])
                    d2p = ps_b.tile([P, 2, T], F32, tag="big")
                    for mc in range(2):
                        for ki in range(2):
                            nc.tensor.matmul(d2p[:, mc, :],
                                             wd2k[ki][:, mc * P:(mc + 1) * P],
                                             d1[:, ki, :], start=(ki == 0),
                                             stop=(ki == 1))
                    d2 = act.tile([P, 2, T], F16, tag="d2")
                    if fl["d2_b0"]:
                        nc.scalar.activation(d2[:], d2p[:], AF.Silu)
                    else:
                        for mc in range(2):
                            nc.scalar.activation(d2[:, mc, :], d2p[:, mc, :],
                                                 AF.Silu, bias=bd2c[:, mc:mc + 1])
                    zp = ps_s.tile([1, T], F32, tag="sm")
                    for ki in range(2):
                        nc.tensor.matmul(zp[:], wdok[ki][:, 0:1], d2[:, ki, :],
                                         start=(ki == 0), stop=(ki == 1))

                    # tail rows: tanh(z/2 + b_do/2), tanh(l_src/2), tanh(l_dst/2)
                    tail = act.tile([3, T], F16, tag="tail")
                    nc.scalar.activation(tail[0:1, :], zp[:], AF.Tanh,
                                         bias=bdoh[:, 0:1], scale=0.5)
                    ts_s = act.tile([1, T], F16, tag="ts_s")
                    nc.scalar.activation(ts_s[:], sg[0:1, 1, :], AF.Tanh,
                                         scale=0.5)
                    ts_d = act.tile([1, T], F16, tag="ts_d")
                    nc.scalar.activation(ts_d[:], dg[0:1, 1, :], AF.Tanh,
                                         scale=0.5)
                    nc.sync.dma_start(tail[1:2, :], ts_s[:])
                    nc.sync.dma_start(tail[2:3, :], ts_d[:])

                    # main head
                    f1p = ps_b.tile([P, 2, T], F32, tag="big")
                    for mc in range(2):
                        msl = slice(mc * P, (mc + 1) * P)
                        nc.tensor.matmul(f1p[:, mc, :], wa[:, msl], sg[:, 0, :],
                                         start=True, stop=False)
                        nc.tensor.matmul(f1p[:, mc, :], wb[:, msl], dg[:, 0, :],
                                         start=False, stop=False)
                        nc.tensor.matmul(f1p[:, mc, :], wc[:, msl], x3[:],
                                         start=False, stop=False)
                        nc.tensor.matmul(f1p[:, mc, :], wtail[:, msl], tail[:],
                                         start=False, stop=True)
                    f1 = act.tile([P, 2, T], F16, tag="f1")
                    for mc in range(2):
                        nc.scalar.activation(f1[:, mc, :], f1p[:, mc, :], AF.Silu,
                                             bias=bf1c[:, mc:mc + 1])
                    f2p = ps_b.tile([P, 2, T], F32, tag="big")
                    for mc in range(2):
                        for ki in range(2):
                            nc.tensor.matmul(f2p[:, mc, :],
                                             wf2k[ki][:, mc * P:(mc + 1) * P],
                                             f1[:, ki, :], start=(ki == 0),
                                             stop=(ki == 1))
                    f2 = act.tile([P, 2, T], F16, tag="f2")
                    if fl["f2_b0"]:
                        nc.scalar.activation(f2[:], f2p[:], AF.Silu)
                    else:
                        for mc in range(2):
                            nc.scalar.activation(f2[:, mc, :], f2p[:, mc, :],
                                                 AF.Silu, bias=bf2c[:, mc:mc + 1])
                    op = ps_s.tile([1, T], F32, tag="sm")
                    for ki in range(2):
                        nc.tensor.matmul(op[:], wbok[ki][:, 0:1], f2[:, ki, :],
                                         start=(ki == 0), stop=(ki == 1))
                    nc.scalar.activation(ob[0:1, sl], op[:], AF.Identity,
                                         bias=bbo1[:, 0:1])

                nc.sync.dma_start(out_v[g:g + 1, :], ob[:])

            if repeats == 1:
                for g in range(G):
                    group_body(g)
            else:
                with tc.For_i(0, repeats, 1):
                    for g in range(G):
                        group_body(g)

    return nc


# ---------------- host-side prep (pure layout/dtype shuffles) ----------------

def wrap_idx(v, e_pad):
    """flat int idx [e_pad] -> [128, e_pad/16] i16 (16-wrap, x8 replicated)."""
    cols = e_pad // 16
    blk = np.asarray(v, np.int64).reshape(cols, 16).T.astype(np.int16)
    return np.ascontiguousarray(np.tile(blk, (8, 1)))


def prep_edge_arrays(src, dst, ea, dual, e_pad):
    e = len(src)
    pad = e_pad - e
    src = np.concatenate([np.asarray(src, np.int64), np.zeros(pad, np.int64)])
    dst = np.concatenate([np.asarray(dst, np.int64), np.zeros(pad, np.int64)])
    ea = np.concatenate(
        [np.asarray(ea, np.float32), np.zeros((pad, D_IN), np.float32)])
    dual = np.concatenate(
        [np.asarray(dual, np.float32), np.zeros((pad, D_DUAL), np.float32)])
    return {
        "ea_t": np.ascontiguousarray(ea.T.astype(np.float16)),
        "dual_t": np.ascontiguousarray(dual.T.astype(np.float16)),
        "src_w": wrap_idx(src, e_pad),
        "dst_w": wrap_idx(dst, e_pad),
    }


def make_common_inputs(inputs):
    node_emb = np.asarray(inputs["node_embedding"], dtype=np.float32)
    logits = np.asarray(inputs["atom_reactivity_logits"], dtype=np.float32)
    aug = np.zeros((N_NODES, AUG_COLS), np.float16)
    aug[:, 0:P] = node_emb.astype(np.float16)
    aug[:, P] = logits.astype(np.float16)
    qconst = np.zeros((128, 3), np.int16)
    qconst[:, 0] = 1          # shift amount
    common = {"aug": aug, "qconst": qconst}
    for name in WEIGHT_NAMES:
        common[name] = np.asarray(inputs[name], dtype=np.float32)
    return common


def make_flags(inputs):
    def z(n):
        return bool(np.all(np.asarray(inputs[n]) == 0.0))

    def one(n):
        return bool(np.all(np.asarray(inputs[n]) == 1.0))

    return {
        "be_b0": z("b_be1") and z("b_be2") and z("b_be3"),
        "be_aff1": one("g_be1") and one("g_be2") and one("g_be3")
                   and z("bb_be1") and z("bb_be2") and z("bb_be3"),
        "d1_b0": z("b_d1"), "d2_b0": z("b_d2"), "f2_b0": z("b_f2"),
    }


def _prep_core_inputs(inputs, core, e_pad):
    e0 = core * E_CORE
    sl = slice(e0, e0 + E_CORE)
    return prep_edge_arrays(
        np.asarray(inputs["edge_index"][0][sl]),
        np.asarray(inputs["edge_index"][1][sl]),
        np.asarray(inputs["edge_attr"][sl]),
        np.asarray(inputs["dual_node_emb"][sl]), e_pad)


def kernel(**inputs):
    from concourse.bass_utils import run_bass_kernel_spmd

    flags = make_flags(inputs)
    nc = build_program(G_FULL, flags=flags)
    nc.finalize()
    common = make_common_inputs(inputs)
    in_maps = []
    for core in range(N_CORES):
        m = dict(common)
        m.update(_prep_core_inputs(inputs, core, E_PAD))
        in_maps.append(m)

    res = run_bass_kernel_spmd(nc, in_maps, list(range(N_CORES)))
    outs = [np.asarray(r["out"])[:E_CORE] for r in res.results]
    return np.concatenate(outs).astype(np.float32)


# revision 4
# speedup vs baseline: 1.5973x; 1.5973x over previous
"""BondReactivityPredictor Trainium2 kernel v2.

Sharding: edges (E=400000) split data-parallel across 8 NeuronCores
(50000/core, padded to 51200 = 25 groups x 2048). node_embedding +
atom logits live in a replicated [N, 256] f16 table ([emb(128) |
logit | pad]); each core fetches the rows for its edge shard with
DMAGatherAnt transpose-gathers (feature-major output, no on-chip
transposes). All matmuls run in f16 (f32 PSUM accumulate); LayerNorm
is computed with mean-centering folded into the weights (W' = W*C so
the matmul output is already centered), a J/128 matmul producing the
broadcast variance directly, and an fp16 quake+Newton rsqrt on DVE
(no activation-table switches: only Silu/Tanh/Square/Identity, all in
the silu table). sigmoid(x) = 0.5 + 0.5*tanh(x/2) is folded into the
f1 weights/bias so tanh rows feed the head directly.
"""

import os
import sys

import numpy as np

for _p in ("/opt/trn_rl_repo", "/root/.axon_site/_ro/trn_rl_repo"):
    if os.path.isdir(_p) and _p not in sys.path:
        sys.path.insert(0, _p)

import concourse.bass as bass
import concourse.bacc as bacc
import concourse.mybir as mybir
import concourse.tile as tile
from concourse import library_config
from concourse.masks import make_identity

F16 = mybir.dt.float16
F32 = mybir.dt.float32
I16 = mybir.dt.int16
AF = mybir.ActivationFunctionType
ALU = mybir.AluOpType
AX = mybir.AxisListType

N_NODES = 25000
D_IN = 16
D_EH = 128
D_H = 256
D_DUAL = 128
P = 128
T = 512                 # edges per macro tile
GROUP_E = 2048          # edges per gather group
MACROS = GROUP_E // T

N_CORES = 8
E_FULL = 400000
E_CORE = E_FULL // N_CORES
G_FULL = 25
E_PAD = G_FULL * GROUP_E

AUG_COLS = 256          # f16: [emb 128 | logit | 127 pad] = 512B rows
EPS = 1e-5
QMAGIC = 22971          # fp16 rsqrt magic: r0 = magic - (bits >> 1)
NEWTON = 1
GATHER_NIDX = 512       # idxs per DMAGatherAnt instr
GATHER_SP = True        # single_packet (safe at 512 idx)

WEIGHT_NAMES = (
    "W_be1", "b_be1", "g_be1", "bb_be1", "W_be2", "b_be2", "g_be2", "bb_be2",
    "W_be3", "b_be3", "g_be3", "bb_be3", "W_d1", "b_d1", "W_d2", "b_d2",
    "W_do", "b_do", "W_f1", "b_f1", "W_f2", "b_f2", "W_bo", "b_bo")


def build_program(G, repeats=1, flags=None, pools=None, fake_gather=False,
                  gather_nidx=None, gather_sp=None, nq=4):
    """Per-core program over G groups of 2048 edges.

    flags: zero/identity detection for bias/affine fast paths (computed
    from the actual inputs by kernel(); all-False is the general path).
    """
    fl = {"be_b0": True, "be_aff1": True, "d1_b0": True, "d2_b0": True,
          "f2_b0": True}
    if flags:
        fl.update(flags)
    po = {"act": 3, "io": 2, "gat": 4, "ps_y": 2, "ps_v": 1, "ps_b": 2,
          "ps_s": 1}
    if pools:
        po.update(pools)
    g_nidx = gather_nidx or GATHER_NIDX
    g_sp = GATHER_SP if gather_sp is None else gather_sp

    nc = bacc.Bacc(num_swdge_queues=nq)
    _qn = [0]

    def next_q():
        q = _qn[0] % nq
        _qn[0] += 1
        return q

    aug = nc.declare_dram_parameter("aug", [N_NODES, AUG_COLS], F16, isOutput=False)
    ea_d = nc.declare_dram_parameter("ea_t", [D_IN, G * GROUP_E], F16, isOutput=False)
    dual_d = nc.declare_dram_parameter("dual_t", [D_DUAL, G * GROUP_E], F16, isOutput=False)
    srcw = nc.declare_dram_parameter("src_w", [128, G * GROUP_E // 16], I16, isOutput=False)
    dstw = nc.declare_dram_parameter("dst_w", [128, G * GROUP_E // 16], I16, isOutput=False)
    qc = nc.declare_dram_parameter("qconst", [128, 3], I16, isOutput=False)

    w_in = {}
    for name, shape in [
        ("W_be1", [D_IN, D_EH]), ("W_be2", [D_EH, D_EH]), ("W_be3", [D_EH, D_EH]),
        ("W_d1", [D_DUAL, D_H]), ("W_d2", [D_H, D_H]), ("W_do", [D_H, 1]),
        ("W_f1", [2 * P + D_EH + 3, D_H]), ("W_f2", [D_H, D_H]), ("W_bo", [D_H, 1]),
        ("b_be1", [D_EH]), ("g_be1", [D_EH]), ("bb_be1", [D_EH]),
        ("b_be2", [D_EH]), ("g_be2", [D_EH]), ("bb_be2", [D_EH]),
        ("b_be3", [D_EH]), ("g_be3", [D_EH]), ("bb_be3", [D_EH]),
        ("b_d1", [D_H]), ("b_d2", [D_H]), ("b_do", [1]),
        ("b_f1", [D_H]), ("b_f2", [D_H]), ("b_bo", [1]),
    ]:
        w_in[name] = nc.declare_dram_parameter(name, shape, F32, isOutput=False)

    out_d = nc.declare_dram_parameter("out", [G * GROUP_E], F32, isOutput=True)
    out_v = out_d.rearrange("(g e) -> g e", e=GROUP_E)

    with tile.TileContext(nc) as tc:
        with (
            tc.tile_pool(name="const", bufs=1) as const,
            tc.tile_pool(name="wst", bufs=1) as wst,
            tc.tile_pool(name="io", bufs=po["io"]) as io,
            tc.tile_pool(name="gat", bufs=po["gat"]) as gat,
            tc.tile_pool(name="act", bufs=po["act"]) as act,
            tc.tile_pool(name="ps_y", bufs=po["ps_y"], space="PSUM") as ps_y,
            tc.tile_pool(name="ps_v", bufs=po["ps_v"], space="PSUM") as ps_v,
            tc.tile_pool(name="ps_b", bufs=po["ps_b"], space="PSUM") as ps_b,
            tc.tile_pool(name="ps_s", bufs=po["ps_s"], space="PSUM") as ps_s,
        ):
            nc.gpsimd.load_library(library_config.mlp)

            ident = const.tile([P, P], F32)
            make_identity(nc, ident[:])
            j128 = const.tile([P, P], F16)
            nc.vector.memset(j128[:], 1.0 / 128)
            qcol = const.tile([128, 3], I16)
            nc.sync.dma_start(qcol[:], qc[:, :])
            qmagic = const.tile([P, T], I16)
            nc.vector.memset(qmagic[:], QMAGIC)
            ones3 = const.tile([3, 1], F32)
            nc.vector.memset(ones3[:], 1.0)

            def cast_load(name, shape, src_ap):
                """f32 DRAM -> f16 SBUF (SWDGE casts in flight)."""
                t = const.tile(shape, F16, name=name)
                nc.gpsimd.dma_start(t[:], src_ap)
                return t

            def center_load(name, K, src_ap):
                """W' = W - rowmean(W): folds LN mean-centering into W."""
                stage = wst.tile([K, P], F32, tag=f"st_{name}")
                nc.sync.dma_start(stage[:], src_ap)
                s = wst.tile([K, 1], F32, tag=f"s_{name}")
                nc.vector.reduce_sum(s[:], stage[:], axis=AX.X)
                m = wst.tile([K, 1], F32, tag=f"m_{name}")
                nc.vector.tensor_scalar_mul(m[:], s[:], 1.0 / 128)
                w = const.tile([K, P], F16, name=name)
                nc.vector.tensor_scalar(
                    out=w[:], in0=stage[:], scalar1=m[:, 0:1], scalar2=None,
                    op0=ALU.subtract)
                return w

            wbe = [center_load(f"wbe{l}", D_IN if l == 1 else D_EH,
                               w_in[f"W_be{l}"][:, :]) for l in (1, 2, 3)]
            wd1 = cast_load("wd1", [P, D_H], w_in["W_d1"][:, :])
            wd2k = [cast_load(f"wd2_{ki}", [P, D_H],
                              w_in["W_d2"][ki * P:(ki + 1) * P, :]) for ki in range(2)]
            wdok = [cast_load(f"wdo_{ki}", [P, 1],
                              w_in["W_do"][ki * P:(ki + 1) * P, :]) for ki in range(2)]
            wa = cast_load("wa", [P, D_H], w_in["W_f1"][0:P, :])
            wb = cast_load("wb", [P, D_H], w_in["W_f1"][P:2 * P, :])
            wc = cast_load("wc", [P, D_H], w_in["W_f1"][2 * P:3 * P, :])
            wf2k = [cast_load(f"wf2_{ki}", [P, D_H],
                              w_in["W_f2"][ki * P:(ki + 1) * P, :]) for ki in range(2)]
            wbok = [cast_load(f"wbo_{ki}", [P, 1],
                              w_in["W_bo"][ki * P:(ki + 1) * P, :]) for ki in range(2)]

            # tail rows (tanh-encoded): weights 0.5*W_f1[384:387];
            # bias fold b_f1'' = b_f1 + 0.5*sum(W_f1[384:387]) as [P,2] col.
            wt_st = wst.tile([3, D_H], F32, tag="wt_st")
            nc.sync.dma_start(wt_st[:], w_in["W_f1"][3 * P:3 * P + 3, :])
            wtail = const.tile([3, D_H], F16, name="wtail")
            nc.vector.tensor_scalar_mul(wtail[:], wt_st[:], 0.5)
            ssum = ps_s.tile([1, D_H], F32, tag="sm")
            nc.tensor.matmul(ssum[:], ones3[:], wt_st[:], start=True, stop=True)
            bf1r = wst.tile([1, D_H], F32, tag="bf1r")
            nc.sync.dma_start(bf1r[:], w_in["b_f1"][None, :])
            bf1pp = wst.tile([1, D_H], F32, tag="bf1pp")
            nc.vector.tensor_scalar_mul(bf1pp[:], ssum[:], 0.5)
            nc.vector.tensor_add(bf1pp[:], bf1pp[:], bf1r[:])
            bf1c = const.tile([P, 2], F32, name="bf1c")
            for mc in range(2):
                tp = ps_s.tile([P, 1], F32, tag="sm")
                nc.tensor.transpose(tp[:], bf1pp[0:1, mc * P:(mc + 1) * P],
                                    ident[0:1, 0:1])
                nc.vector.tensor_copy(bf1c[:, mc:mc + 1], tp[:])

            def col2(name):
                t = const.tile([P, 2], F32, name=f"c_{name}")
                nc.sync.dma_start(t[:], w_in[name][:].rearrange("(mc d) -> d mc", d=P))
                return t

            bd1c = None if fl["d1_b0"] else col2("b_d1")
            bd2c = None if fl["d2_b0"] else col2("b_d2")
            bf2c = None if fl["f2_b0"] else col2("b_f2")

            def col1(name):
                t = const.tile([P, 1], F32, name=f"c_{name}")
                nc.sync.dma_start(t[:], w_in[name][:, None])
                return t

            becols = {}
            if not fl["be_aff1"]:
                for l in (1, 2, 3):
                    becols[("g", l)] = col1(f"g_be{l}")
                    becols[("bb", l)] = col1(f"bb_be{l}")
            if not fl["be_b0"]:
                # b' = b - mean(b) column, per layer
                onec = const.tile([P, 1], F32, name="onec")
                nc.vector.memset(onec[:], 1.0)
                oner = const.tile([1, P], F32, name="oner")
                nc.vector.memset(oner[:], 1.0)
                for l in (1, 2, 3):
                    bcol = col1(f"b_be{l}")
                    s11 = ps_s.tile([1, 1], F32, tag="sm")
                    nc.tensor.matmul(s11[:], bcol[:], onec[:], start=True, stop=True)
                    sc = wst.tile([1, 1], F32, tag=f"bs_{l}")
                    nc.vector.tensor_scalar_mul(sc[:], s11[:], 1.0 / 128)
                    mb = ps_s.tile([P, 1], F32, tag="sm")
                    nc.tensor.matmul(mb[:], oner[:], sc[:], start=True, stop=True)
                    bp = const.tile([P, 1], F32, name=f"bp_{l}")
                    nc.vector.tensor_sub(bp[:], bcol[:], mb[:])
                    becols[("b", l)] = bp

            bdoh = const.tile([1, 1], F32, name="bdoh")
            tmp11 = wst.tile([1, 1], F32, tag="t11")
            nc.sync.dma_start(tmp11[:], w_in["b_do"][None, :])
            nc.vector.tensor_scalar_mul(bdoh[:], tmp11[:], 0.5)
            bbo1 = const.tile([1, 1], F32, name="bbo1")
            nc.sync.dma_start(bbo1[:], w_in["b_bo"][None, :])

            # ---------------- main loop ----------------
            IDX_COLS = g_nidx // 16
            sit_all = const.tile([128, G * GROUP_E // 16], I16, name="sit_all")
            nc.sync.dma_start(sit_all[:], srcw[:, :])
            dit_all = const.tile([128, G * GROUP_E // 16], I16, name="dit_all")
            nc.sync.dma_start(dit_all[:], dstw[:, :])

            def ln_layer(l, w, x_rhs):
                yc = ps_y.tile([P, T], F32, tag="y")
                nc.tensor.matmul(yc[:], w[:], x_rhs, start=True, stop=True)
                ysb = act.tile([P, T], F16, tag="ysb")
                if fl["be_b0"]:
                    nc.vector.tensor_copy(ysb[:], yc[:])
                else:
                    nc.vector.tensor_scalar_add(ysb[:], yc[:],
                                                becols[("b", l)][:, 0:1])
                ysrc = ysb
                sq = act.tile([P, T], F16, tag="sq")
                nc.vector.tensor_mul(sq[:], ysrc[:], ysrc[:])
                var = ps_v.tile([P, T], F32, tag="v")
                nc.tensor.matmul(var[:], j128[:], sq[:], start=True, stop=True)
                v16 = act.tile([P, T], F16, tag="v16")
                nc.vector.tensor_scalar_add(v16[:], var[:], EPS)
                qt = act.tile([P, T], I16, tag="qt")
                nc.vector.tensor_scalar(
                    out=qt[:], in0=v16[:].bitcast(I16), scalar1=qcol[:, 0:1],
                    scalar2=None, op0=ALU.logical_shift_right)
                r = act.tile([P, T], F16, tag="qr")
                nc.vector.tensor_tensor(
                    out=r[:].bitcast(I16), in0=qmagic[:], in1=qt[:],
                    op=ALU.subtract)
                for _ in range(NEWTON):
                    rr = act.tile([P, T], F16, tag="qrr")
                    nc.vector.tensor_mul(rr[:], r[:], r[:])
                    nc.vector.tensor_mul(rr[:], rr[:], v16[:])
                    nc.vector.tensor_scalar(
                        out=rr[:], in0=rr[:], scalar1=-0.5, scalar2=1.5,
                        op0=ALU.mult, op1=ALU.add)
                    r2 = act.tile([P, T], F16, tag="qr2")
                    nc.vector.tensor_mul(r2[:], r[:], rr[:])
                    r = r2
                xn = act.tile([P, T], F16, tag="xn")
                nc.vector.tensor_mul(xn[:], ysrc[:], r[:])
                x = act.tile([P, T], F16, tag=f"x{l}")
                if fl["be_aff1"]:
                    nc.scalar.activation(x[:], xn[:], AF.Silu)
                else:
                    nc.scalar.activation(x[:], xn[:], AF.Silu,
                                         bias=becols[("bb", l)][:, 0:1],
                                         scale=becols[("g", l)][:, 0:1])
                return x

            def group_body(g):
                gcs = GROUP_E // 16
                sit = sit_all[:, g * gcs:(g + 1) * gcs]
                dit = dit_all[:, g * gcs:(g + 1) * gcs]
                ea_t = io.tile([D_IN, GROUP_E], F16, tag="ea")
                nc.sync.dma_start(ea_t[:], ea_d[:, g * GROUP_E:(g + 1) * GROUP_E])
                du_t = io.tile([D_DUAL, GROUP_E], F16, tag="du")
                nc.sync.dma_start(du_t[:], dual_d[:, g * GROUP_E:(g + 1) * GROUP_E])
                ob = io.tile([1, GROUP_E], F32, tag="ob")

                gpg = GROUP_E // g_nidx
                sgs = []
                dgs = []
                for q in range(gpg):
                    ics = slice(q * IDX_COLS, (q + 1) * IDX_COLS)
                    st = gat.tile([128, 2, g_nidx], F16, tag="sg")
                    dt_ = gat.tile([128, 2, g_nidx], F16, tag="dg")
                    if fake_gather:
                        fsrc = dual_d[:, 0:2 * g_nidx].rearrange(
                            "p (c e) -> p c e", c=2)
                        nc.sync.dma_start(st[:], fsrc)
                        nc.sync.dma_start(dt_[:], fsrc)
                    else:
                        nc.gpsimd.dma_gather(
                            st[:], aug[:, :], sit[:, ics], g_nidx, g_nidx,
                            AUG_COLS, transpose=True, single_packet=g_sp,
                            queue_num=next_q())
                        nc.gpsimd.dma_gather(
                            dt_[:], aug[:, :], dit[:, ics], g_nidx, g_nidx,
                            AUG_COLS, transpose=True, single_packet=g_sp,
                            queue_num=next_q())
                    sgs.append(st)
                    dgs.append(dt_)

                for mi in range(MACROS):
                    sl = slice(mi * T, (mi + 1) * T)
                    q = (mi * T) // g_nidx
                    off = (mi * T) % g_nidx
                    gsl = slice(off, off + T)
                    sgt_, dgt_ = sgs[q], dgs[q]

                    # BondEmbedding chain
                    x = ea_t[:, sl]
                    for l in (1, 2, 3):
                        x = ln_layer(l, wbe[l - 1], x)
                    x3 = x

                    # dual chain
                    d1p = ps_b.tile([P, 2, T], F32, tag="big")
                    for mc in range(2):
                        nc.tensor.matmul(d1p[:, mc, :], wd1[:, mc * P:(mc + 1) * P],
                                         du_t[:, sl], start=True, stop=True)
                    d1 = act.tile([P, 2, T], F16, tag="d1")
                    if fl["d1_b0"]:
                        nc.scalar.activation(d1[:], d1p[:], AF.Silu)
                    else:
                        for mc in range(2):
                            nc.scalar.activation(d1[:, mc, :], d1p[:, mc, :],
                                                 AF.Silu, bias=bd1c[:, mc:mc + 1])
                    d2p = ps_b.tile([P, 2, T], F32, tag="big")
                    for mc in range(2):
                        for ki in range(2):
                            nc.tensor.matmul(d2p[:, mc, :],
                                             wd2k[ki][:, mc * P:(mc + 1) * P],
                                             d1[:, ki, :], start=(ki == 0),
                                             stop=(ki == 1))
                    d2 = act.tile([P, 2, T], F16, tag="d2")
                    if fl["d2_b0"]:
                        nc.scalar.activation(d2[:], d2p[:], AF.Silu)
                    else:
                        for mc in range(2):
                            nc.scalar.activation(d2[:, mc, :], d2p[:, mc, :],
                                                 AF.Silu, bias=bd2c[:, mc:mc + 1])
                    zp = ps_s.tile([1, T], F32, tag="sm")
                    for ki in range(2):
                        nc.tensor.matmul(zp[:], wdok[ki][:, 0:1], d2[:, ki, :],
                                         start=(ki == 0), stop=(ki == 1))

                    # tail rows: tanh(z/2 + b_do/2), tanh(l_src/2), tanh(l_dst/2)
                    tail = act.tile([3, T], F16, tag="tail")
                    nc.scalar.activation(tail[0:1, :], zp[:], AF.Tanh,
                                         bias=bdoh[:, 0:1], scale=0.5)
                    ts_s = act.tile([1, T], F16, tag="ts_s")
                    nc.scalar.activation(ts_s[:], sgt_[0:1, 1, gsl], AF.Tanh,
                                         scale=0.5)
                    ts_d = act.tile([1, T], F16, tag="ts_d")
                    nc.scalar.activation(ts_d[:], dgt_[0:1, 1, gsl], AF.Tanh,
                                         scale=0.5)
                    nc.sync.dma_start(tail[1:2, :], ts_s[:])
                    nc.sync.dma_start(tail[2:3, :], ts_d[:])

                    # main head
                    f1p = ps_b.tile([P, 2, T], F32, tag="big")
                    for mc in range(2):
                        msl = slice(mc * P, (mc + 1) * P)
                        nc.tensor.matmul(f1p[:, mc, :], wa[:, msl],
                                         sgt_[:, 0, gsl], start=True, stop=False)
                        nc.tensor.matmul(f1p[:, mc, :], wb[:, msl],
                                         dgt_[:, 0, gsl], start=False, stop=False)
                        nc.tensor.matmul(f1p[:, mc, :], wc[:, msl], x3[:],
                                         start=False, stop=False)
                        nc.tensor.matmul(f1p[:, mc, :], wtail[:, msl], tail[:],
                                         start=False, stop=True)
                    f1 = act.tile([P, 2, T], F16, tag="f1")
                    for mc in range(2):
                        nc.scalar.activation(f1[:, mc, :], f1p[:, mc, :], AF.Silu,
                                             bias=bf1c[:, mc:mc + 1])
                    f2p = ps_b.tile([P, 2, T], F32, tag="big")
                    for mc in range(2):
                        for ki in range(2):
                            nc.tensor.matmul(f2p[:, mc, :],
                                             wf2k[ki][:, mc * P:(mc + 1) * P],
                                             f1[:, ki, :], start=(ki == 0),
                                             stop=(ki == 1))
                    f2 = act.tile([P, 2, T], F16, tag="f2")
                    if fl["f2_b0"]:
                        nc.scalar.activation(f2[:], f2p[:], AF.Silu)
                    else:
                        for mc in range(2):
                            nc.scalar.activation(f2[:, mc, :], f2p[:, mc, :],
                                                 AF.Silu, bias=bf2c[:, mc:mc + 1])
                    op = ps_s.tile([1, T], F32, tag="sm")
                    for ki in range(2):
                        nc.tensor.matmul(op[:], wbok[ki][:, 0:1], f2[:, ki, :],
                                         start=(ki == 0), stop=(ki == 1))
                    nc.scalar.activation(ob[0:1, sl], op[:], AF.Identity,
                                         bias=bbo1[:, 0:1])

                nc.sync.dma_start(out_v[g:g + 1, :], ob[:])

            if repeats == 1:
                for g in range(G):
                    group_body(g)
            else:
                with tc.For_i(0, repeats, 1):
                    for g in range(G):
                        group_body(g)

    return nc


# ---------------- host-side prep (pure layout/dtype shuffles) ----------------

def wrap_idx(v, e_pad):
    """flat int idx [e_pad] -> [128, e_pad/16] i16 (16-wrap, x8 replicated)."""
    cols = e_pad // 16
    blk = np.asarray(v, np.int64).reshape(cols, 16).T.astype(np.int16)
    return np.ascontiguousarray(np.tile(blk, (8, 1)))


def prep_edge_arrays(src, dst, ea, dual, e_pad):
    e = len(src)
    pad = e_pad - e
    src = np.concatenate([np.asarray(src, np.int64), np.zeros(pad, np.int64)])
    dst = np.concatenate([np.asarray(dst, np.int64), np.zeros(pad, np.int64)])
    ea = np.concatenate(
        [np.asarray(ea, np.float32), np.zeros((pad, D_IN), np.float32)])
    dual = np.concatenate(
        [np.asarray(dual, np.float32), np.zeros((pad, D_DUAL), np.float32)])
    return {
        "ea_t": np.ascontiguousarray(ea.T.astype(np.float16)),
        "dual_t": np.ascontiguousarray(dual.T.astype(np.float16)),
        "src_w": wrap_idx(src, e_pad),
        "dst_w": wrap_idx(dst, e_pad),
    }


def make_common_inputs(inputs):
    node_emb = np.asarray(inputs["node_embedding"], dtype=np.float32)
    logits = np.asarray(inputs["atom_reactivity_logits"], dtype=np.float32)
    aug = np.zeros((N_NODES, AUG_COLS), np.float16)
    aug[:, 0:P] = node_emb.astype(np.float16)
    aug[:, P] = logits.astype(np.float16)
    qconst = np.zeros((128, 3), np.int16)
    qconst[:, 0] = 1          # shift amount
    common = {"aug": aug, "qconst": qconst}
    for name in WEIGHT_NAMES:
        common[name] = np.asarray(inputs[name], dtype=np.float32)
    return common


def make_flags(inputs):
    def z(n):
        return bool(np.all(np.asarray(inputs[n]) == 0.0))

    def one(n):
        return bool(np.all(np.asarray(inputs[n]) == 1.0))

    return {
        "be_b0": z("b_be1") and z("b_be2") and z("b_be3"),
        "be_aff1": one("g_be1") and one("g_be2") and one("g_be3")
                   and z("bb_be1") and z("bb_be2") and z("bb_be3"),
        "d1_b0": z("b_d1"), "d2_b0": z("b_d2"), "f2_b0": z("b_f2"),
    }


def _prep_core_inputs(inputs, core, e_pad):
    e0 = core * E_CORE
    sl = slice(e0, e0 + E_CORE)
    return prep_edge_arrays(
        np.asarray(inputs["edge_index"][0][sl]),
        np.asarray(inputs["edge_index"][1][sl]),
        np.asarray(inputs["edge_attr"][sl]),
        np.asarray(inputs["dual_node_emb"][sl]), e_pad)


def kernel(**inputs):
    from concourse.bass_utils import run_bass_kernel_spmd

    flags = make_flags(inputs)
    nc = build_program(G_FULL, flags=flags)
    nc.finalize()
    common = make_common_inputs(inputs)
    in_maps = []
    for core in range(N_CORES):
        m = dict(common)
        m.update(_prep_core_inputs(inputs, core, E_PAD))
        in_maps.append(m)

    res = run_bass_kernel_spmd(nc, in_maps, list(range(N_CORES)))
    outs = [np.asarray(r["out"])[:E_CORE] for r in res.results]
    return np.concatenate(outs).astype(np.float32)
